# revision 9
# baseline (speedup 1.0000x reference)
"""Trainium2 Bass kernel for nn_AttrSoftLoss (masked multilabel soft-margin loss).

Reference semantics: per row, drop the k = round(0.95 * n_zero) zero-labeled
positions whose fixed uniform draws (jax.random.key(42)) are smallest, then
average  -[a*log_sigmoid(s) + (1-a)*log_sigmoid(-s)]  over kept positions;
mean over rows.  With g = 1-2a and x = g*s this is
loss = [sum_kept softplus(x)] / (B*C)  (the mask keeps all a=1 positions).

Host prep (layout/encoding only): rows pre-permuted into ascending order of
the fixed input-independent uniform matrix (the dropped set becomes "the
first k zero-labeled entries" in storage order), data stored TRANSPOSED
(classes on partitions, rows on the free dim), scores cast to fp16, labels
recoded as g = 1-2a in {+1,-1} fp16.

Device math per [128, 1024] class-block cb: the keep decision
c > rint(0.95*nz) (c = inclusive zero-prefix count, nz = row zero count)
is evaluated in the integer-exact scaled form
    Q = 20c + 20*1025*a - 19*nz - 10.4 > 0
(deviates from round-half-even only on ~234 of 8.4M boundary elements,
rel err 5e-5, numpy-verified).  In g-units every data term is linear:
    q_psum = W @ g_cb + L_cb @ cs            (PE; f32-exact by construction)
    W[k,i] = 10*[k<=i], diag -10240          (own-block prefix + ones-pusher)
    cs[b,j] = sum_k g_b[k,j]                 (GpSimd partition-reduce)
    L_cb[b] = +0.5 if b<cb else -9.5         (cross-block prefix + 0.95*nz)
    kept <=> q_psum > thr[i,cb] = -1280*cb - 10*(i+1) - 511.6 (host f32 const)
and the whole mask+multiply+reduce is ONE fused DVE op per block:
    stt(scr, q_psum, thr_ptr, sp, is_gt, mult, accum_out=stats).
No prefix scan (v1: 2.7us/block on DVE), no chain adds (v3): the
cross-block counts ride entirely on the idle PE/GpSimd engines.

ScalarE computes softplus(x) = Ln(1 + Exp(x)) in fp16 over [128, 2048]
chunks (Exp and Ln share one act table -> 2 table loads total); DVE runs
x = g*s (2x tt) and the stt.  Final per-core scalar via a tiny f32 matmul.
Batch sharded 1024 rows/core; host sums the 8 partials at gather time (a
4-byte device AllReduce costs ~50us + NEFF barrier, dominating the kernel).
"""

import numpy as np

B, C = 8192, 1024
N_CORES = 8
ROWS = B // N_CORES  # 1024 rows per core (free dim after transpose)
NB = C // 128        # 8 class-blocks per core (partition dim)

_cache: dict = {}


def _build_nc():
    from concourse import bacc, mybir, tile

    Alu = mybir.AluOpType
    Act = mybir.ActivationFunctionType
    f32 = mybir.dt.float32
    f16 = mybir.dt.float16

    nc = bacc.Bacc(
        "TRN2", target_bir_lowering=False, debug=False, num_devices=N_CORES
    )
    s_d = nc.dram_tensor("s", [C, ROWS], f16, kind="ExternalInput")
    g_d = nc.dram_tensor("g", [C, ROWS], f16, kind="ExternalInput")
    w_d = nc.dram_tensor("wtri", [128, 128], f16, kind="ExternalInput")
    l_d = nc.dram_tensor("lcoef", [NB * NB, 128], f16, kind="ExternalInput")
    thr_d = nc.dram_tensor("thr", [128, NB], f32, kind="ExternalInput")
    out_d = nc.dram_tensor("out", [1, 1], f32, kind="ExternalOutput")

    with tile.TileContext(nc) as tc:
        with (
            tc.tile_pool(name="io", bufs=1) as io,
            tc.tile_pool(name="work", bufs=3) as work,
            tc.tile_pool(name="stat", bufs=1) as stat,
            tc.tile_pool(name="psum", bufs=3, space="PSUM") as psum,
            tc.tile_pool(name="psum_out", bufs=1, space="PSUM") as psum_out,
        ):
            wtri = stat.tile([128, 128], f16)
            thr = stat.tile([128, NB], f32)
            stats = stat.tile([128, NB], f32)
            nc.sync.dma_start(out=wtri[:], in_=w_d[:, :])
            nc.sync.dma_start(out=thr[:], in_=thr_d[:, :])
            # per-block cs-coefficient lhsTs; separate tiles so each sits at
            # base partition 0 (matmul lhsT requirement)
            lcoef = []
            for cb in range(NB):
                lc = stat.tile([NB, 128], f16, tag=f"lc{cb}")
                nc.sync.dma_start(out=lc[:], in_=l_d[NB * cb : NB * (cb + 1), :])
                lcoef.append(lc)

            g_big = stat.tile([128, NB * ROWS], f16)
            s_big = stat.tile([128, NB * ROWS], f16)
            x_big = stat.tile([128, NB * ROWS], f16)
            ex_big = stat.tile([128, NB * ROWS], f16)
            sp_big = stat.tile([128, NB * ROWS], f16)
            cs = stat.tile([NB, ROWS], f16)

            def blk(t, cb):
                return t[:, ROWS * cb : ROWS * (cb + 1)]

            # Interleave g/s block loads; g gates the colsums (cs barrier),
            # s gates the ACT stream.
            for cb in range(NB):
                nc.sync.dma_start(
                    out=blk(g_big, cb), in_=g_d[128 * cb : 128 * (cb + 1), :]
                )
                nc.sync.dma_start(
                    out=blk(s_big, cb), in_=s_d[128 * cb : 128 * (cb + 1), :]
                )

            # Per-block column sums on GpSimd (partition-reduce, idle engine).
            # GpSimd ISA outs must sit at base partition 0: reduce into a flat
            # [1, NB*ROWS] staging row, then restack to [NB, ROWS] via DMA.
            cs_flat = stat.tile([1, NB * ROWS], f16)
            with nc.allow_low_precision(
                reason="cs sums 128 values of +-1: exact integers in fp16"
            ):
                for cb in range(NB):
                    nc.gpsimd.tensor_reduce(
                        blk(cs_flat, cb), blk(g_big, cb),
                        mybir.AxisListType.C, Alu.add,
                    )
            for cb in range(NB):
                nc.sync.dma_start(out=cs[cb : cb + 1, :], in_=blk(cs_flat, cb))

            # x = g*s and softplus(x) in [128, 2048] chunks.
            NCH = 4
            CW = NB * ROWS // NCH
            for ch in range(NCH):
                sl = slice(CW * ch, CW * (ch + 1))
                nc.vector.tensor_tensor(
                    x_big[:, sl], g_big[:, sl], s_big[:, sl], Alu.mult
                )
                nc.scalar.activation(ex_big[:, sl], x_big[:, sl], Act.Exp)
                nc.scalar.activation(
                    sp_big[:, sl], ex_big[:, sl], Act.Ln, bias=1.0
                )

            for cb in range(NB):
                q = psum.tile([128, ROWS], f32, tag="q")
                for h in range(2):
                    sl = slice(512 * h, 512 * (h + 1))
                    nc.tensor.matmul(
                        q[:, sl], wtri[:],
                        g_big[:, ROWS * cb + 512 * h : ROWS * cb + 512 * (h + 1)],
                        start=True, stop=False,
                    )
                    nc.tensor.matmul(
                        q[:, sl], lcoef[cb][:], cs[:, sl],
                        start=False, stop=True,
                    )
                scr = work.tile([128, ROWS], f16, tag="scr")
                nc.vector.scalar_tensor_tensor(
                    scr[:], q[:], thr[:, cb : cb + 1], blk(sp_big, cb),
                    op0=Alu.is_gt, op1=Alu.mult,
                    accum_out=stats[:, cb : cb + 1],
                )

            acc = stat.tile([128, 1], f32)
            nc.vector.tensor_reduce(
                acc[:], stats[:], mybir.AxisListType.X, Alu.add
            )
            ones_a = stat.tile([128, 1], f32)
            nc.vector.memset(ones_a[:], 1.0 / (B * C))
            part = psum_out.tile([1, 1], f32)
            nc.tensor.matmul(part[:], ones_a[:], acc[:], start=True, stop=True)
            res = stat.tile([1, 1], f32)
            nc.vector.tensor_copy(res[:], part[:])
            nc.sync.dma_start(out=out_d[:, :], in_=res[:])

    nc.compile()
    return nc


def _get_nc():
    if "nc" not in _cache:
        _cache["nc"] = _build_nc()
    return _cache["nc"]


def _get_perm():
    """Constant per-row ascending-argsort of the fixed uniform matrix."""
    if "perm" not in _cache:
        import jax

        with jax.default_device(jax.devices("cpu")[0]):
            u = np.asarray(jax.random.uniform(jax.random.key(42), (B, C)))
        _cache["perm"] = np.argsort(u, axis=1, kind="stable")
    return _cache["perm"]


def _consts():
    if "wtri" not in _cache:
        # lhsT[k,i] = 10*[k<=i], diag -10240  (matmul computes lhsT.T @ rhs)
        w = 10.0 * np.triu(np.ones((128, 128), np.float32))
        np.fill_diagonal(w, -10240.0)
        _cache["wtri"] = w.astype(np.float16)
        lc = np.zeros((NB * NB, 128), np.float32)
        for cb in range(NB):
            coef = np.where(np.arange(NB) < cb, 0.5, -9.5)
            lc[NB * cb : NB * (cb + 1), :] = coef[:, None]
        _cache["lcoef"] = lc.astype(np.float16)
        i = np.arange(128, dtype=np.float64)[:, None]
        cb = np.arange(NB, dtype=np.float64)[None, :]
        thr = -1280.0 * cb - 10.0 * (i + 1.0) - 511.6
        _cache["thr"] = thr.astype(np.float32)
    return _cache["wtri"], _cache["lcoef"], _cache["thr"]


def _make_in_maps(scores: np.ndarray, attributes: np.ndarray):
    perm = _get_perm()
    s_p = np.take_along_axis(np.asarray(scores, dtype=np.float32), perm, axis=1)
    a_p = np.take_along_axis(np.asarray(attributes, dtype=np.int32), perm, axis=1)
    s16 = s_p.astype(np.float16)
    g16 = (1 - 2 * a_p).astype(np.float16)
    wtri, lcoef, thr = _consts()
    in_maps = []
    for i in range(N_CORES):
        r0, r1 = i * ROWS, (i + 1) * ROWS
        in_maps.append(
            {
                "s": np.ascontiguousarray(s16[r0:r1].T),
                "g": np.ascontiguousarray(g16[r0:r1].T),
                "wtri": wtri,
                "lcoef": lcoef,
                "thr": thr,
            }
        )
    return in_maps


def _run(in_maps, trace=False, **kwargs):
    from concourse import bass_utils

    return bass_utils.run_bass_kernel_spmd(
        _get_nc(), in_maps, core_ids=list(range(N_CORES)), trace=trace, **kwargs
    )


def kernel(scores: np.ndarray, attributes: np.ndarray) -> np.ndarray:
    res = _run(_make_in_maps(scores, attributes))
    parts = np.stack(
        [np.asarray(r["out"], dtype=np.float32).reshape(()) for r in res.results]
    )
    return np.float32(np.sum(parts, dtype=np.float32)).reshape(())[()]


# revision 12
# speedup vs baseline: 23.5266x; 23.5266x over previous
"""Trainium2 Bass kernel for nn_AttrSoftLoss (masked multilabel soft-margin loss).

Reference semantics: per row, drop the k = round(0.95 * n_zero) zero-labeled
positions whose fixed uniform draws (jax.random.key(42)) are smallest, then
average  -[a*log_sigmoid(s) + (1-a)*log_sigmoid(-s)]  over kept positions;
mean over rows.  With g = 1-2a and x = g*s this is
loss = [sum_kept softplus(x)] / (B*C)  (the mask keeps all a=1 positions).

Host prep (layout/encoding only): rows pre-permuted into ascending order of
the fixed input-independent uniform matrix (the dropped set becomes "the
first k zero-labeled entries" in storage order), data stored TRANSPOSED
(classes on partitions, rows on the free dim), scores cast to fp16, labels
recoded as gg = 10*(1-2a) in {+10,-10} fp16 (the 10x lets every derived
count stay integer-exact in fp16 and folds into Exp's scale immediate).

Device math per [128, 1024] class-block cb: the keep decision
c > rint(0.95*nz) (c = inclusive zero-prefix count, nz = row zero count) is
evaluated in the integer-exact scaled form Q = 20c + 20*1025*a - 19*nz -
10.4 > 0 (deviates from round-half-even only on ~234 of 8.4M boundary
elements, rel err 5e-5, numpy-verified).  In gg-units all data terms are
linear, giving per block two PE matmuls and one fused DVE op:

    q_psum = W @ gg_cb + J @ V_cb                      (PE, f32-exact)
    W[k,i] = [k<=i], diag -1024    (own-block prefix + ones-pusher)
    V_cb   = -0.95*GT + sum_{b<cb} gg_b                (DVE fp16 chain; GT =
             sum_b gg_b; all values half-integers, fp16-exact)
    kept  <=> q_psum > thr[i,cb] = -1280*cb - 10*(i+1) - 511.6  (f32 const)
    stt(scr, q_psum, thr_ptr, sp, is_gt, mult, accum_out=stats)

No prefix scan (2.7us/block on DVE, v1) and no GpSimd cross-lane reduce
(127us/block on HW!, v4): cross-block counts ride on fp16 chain adds (2x
DVE mode) contracted by the all-ones J matmul on the PE.

ScalarE computes softplus(x) = Ln(1 + Exp(0.1*xx)), xx = gg*s, over
[128, 2048] chunks; the act-table list passed to insert_act_table_loads is
pruned (order-preserving, so runtime set ids stay valid) so Exp and Ln both
resolve to natural_log_exp_and_others: one table load total.
Batch is sharded 1024 rows/core (pure data parallel); the host sums the 8
partial scalars at gather time (a 4-byte device AllReduce costs ~50us + a
~100us NEFF entry barrier, dominating the whole kernel).
"""

import numpy as np

B, C = 8192, 1024
N_CORES = 8
ROWS = B // N_CORES  # 1024 rows per core (free dim after transpose)
NB = C // 128        # 8 class-blocks per core (partition dim)

_cache: dict = {}


def _make_bacc():
    from concourse import bacc, mybir

    class PrunedTableBacc(bacc.Bacc):
        """Prune Exp/Ln from every act-table set except
        natural_log_exp_and_others (order preserved, so the emitted
        act_func_set_id still indexes the real act_info list) - forces the
        first-fit chooser to put Exp and Ln on the one shared table."""

        def insert_act_table_loads(self):
            import bass_rust as _bass_rust
            from concourse.hw_specs import get_activation_tables

            keep = "natural_log_exp_and_others"
            drop = {
                mybir.ActivationFunctionType.Exp,
                mybir.ActivationFunctionType.Ln,
            }
            tables = []
            for name, funcs in get_activation_tables(self.m.arch).items():
                if name != keep:
                    funcs = {f for f in funcs if f not in drop}
                tables.append((name, funcs))
            _bass_rust.insert_act_table_loads(self, tables)

    return PrunedTableBacc(
        "TRN2", target_bir_lowering=False, debug=False, num_devices=N_CORES
    )


def _build_nc():
    from concourse import mybir, tile

    Alu = mybir.AluOpType
    Act = mybir.ActivationFunctionType
    f32 = mybir.dt.float32
    f16 = mybir.dt.float16

    nc = _make_bacc()
    s_d = nc.dram_tensor("s", [C, ROWS], f16, kind="ExternalInput")
    g_d = nc.dram_tensor("gg", [C, ROWS], f16, kind="ExternalInput")
    w_d = nc.dram_tensor("wtri", [128, 128], f16, kind="ExternalInput")
    thr_d = nc.dram_tensor("thr", [128, NB], f32, kind="ExternalInput")
    out_d = nc.dram_tensor("out", [1, 1], f32, kind="ExternalOutput")

    with tile.TileContext(nc) as tc:
        with (
            tc.tile_pool(name="work", bufs=3) as work,
            tc.tile_pool(name="stat", bufs=1) as stat,
            tc.tile_pool(name="psum", bufs=3, space="PSUM") as psum,
            tc.tile_pool(name="psum_out", bufs=1, space="PSUM") as psum_out,
        ):
            wtri = stat.tile([128, 128], f16)
            thr = stat.tile([128, NB], f32)
            stats = stat.tile([128, NB], f32)
            jmat = stat.tile([128, 128], f16)
            nc.sync.dma_start(out=wtri[:], in_=w_d[:, :])
            nc.sync.dma_start(out=thr[:], in_=thr_d[:, :])
            nc.vector.memset(jmat[:], 1.0)

            g_big = stat.tile([128, NB * ROWS], f16)
            s_big = stat.tile([128, NB * ROWS], f16)
            x_big = stat.tile([128, NB * ROWS], f16)
            ex_big = stat.tile([128, NB * ROWS], f16)
            sp_big = stat.tile([128, NB * ROWS], f16)

            def blk(t, cb):
                return t[:, ROWS * cb : ROWS * (cb + 1)]

            # Interleave gg/s block loads: gg gates chains+matmuls, s the ACT.
            for cb in range(NB):
                nc.sync.dma_start(
                    out=blk(g_big, cb), in_=g_d[128 * cb : 128 * (cb + 1), :]
                )
                nc.sync.dma_start(
                    out=blk(s_big, cb), in_=s_d[128 * cb : 128 * (cb + 1), :]
                )

            # GT = sum_b gg_b as a pair tree (depth 3, 7 tts).
            p01 = stat.tile([128, ROWS], f16)
            p23 = stat.tile([128, ROWS], f16)
            p45 = stat.tile([128, ROWS], f16)
            p67 = stat.tile([128, ROWS], f16)
            nc.vector.tensor_tensor(p01[:], blk(g_big, 0), blk(g_big, 1), Alu.add)
            nc.vector.tensor_tensor(p23[:], blk(g_big, 2), blk(g_big, 3), Alu.add)
            nc.vector.tensor_tensor(p45[:], blk(g_big, 4), blk(g_big, 5), Alu.add)
            nc.vector.tensor_tensor(p67[:], blk(g_big, 6), blk(g_big, 7), Alu.add)
            p03 = stat.tile([128, ROWS], f16)
            p47 = stat.tile([128, ROWS], f16)
            nc.vector.tensor_tensor(p03[:], p01[:], p23[:], Alu.add)
            nc.vector.tensor_tensor(p47[:], p45[:], p67[:], Alu.add)
            gt = stat.tile([128, ROWS], f16)
            nc.vector.tensor_tensor(gt[:], p03[:], p47[:], Alu.add)

            # V chain: V_0 = -0.95*GT (exact half-integers), V_{cb+1} = V + gg.
            V = [None] * NB
            v0 = stat.tile([128, ROWS], f16, tag="V0")
            nc.vector.tensor_scalar(v0[:], gt[:], -0.95, None, Alu.mult)
            V[0] = v0
            for cb in range(1, NB):
                nxt = stat.tile([128, ROWS], f16, tag=f"V{cb}")
                nc.vector.tensor_tensor(
                    nxt[:], V[cb - 1][:], blk(g_big, cb - 1), Alu.add
                )
                V[cb] = nxt

            # xx = gg*s; softplus(x) = Ln(1 + Exp(0.1*xx)) in [128,2048] chunks
            NCH = 4
            CW = NB * ROWS // NCH
            for ch in range(NCH):
                sl = slice(CW * ch, CW * (ch + 1))
                nc.vector.tensor_tensor(
                    x_big[:, sl], g_big[:, sl], s_big[:, sl], Alu.mult
                )
                nc.scalar.activation(ex_big[:, sl], x_big[:, sl], Act.Exp, scale=0.1)
                nc.scalar.activation(
                    sp_big[:, sl], ex_big[:, sl], Act.Ln, bias=1.0
                )

            for cb in range(NB):
                q = psum.tile([128, ROWS], f32, tag="q")
                for h in range(2):
                    sl = slice(512 * h, 512 * (h + 1))
                    nc.tensor.matmul(
                        q[:, sl], wtri[:],
                        g_big[:, ROWS * cb + 512 * h : ROWS * cb + 512 * (h + 1)],
                        start=True, stop=False,
                    )
                    nc.tensor.matmul(
                        q[:, sl], jmat[:], V[cb][:, sl], start=False, stop=True,
                    )
                scr = work.tile([128, ROWS], f16, tag="scr")
                nc.vector.scalar_tensor_tensor(
                    scr[:], q[:], thr[:, cb : cb + 1], blk(sp_big, cb),
                    op0=Alu.is_gt, op1=Alu.mult,
                    accum_out=stats[:, cb : cb + 1],
                )

            acc = stat.tile([128, 1], f32)
            nc.vector.tensor_reduce(
                acc[:], stats[:], mybir.AxisListType.X, Alu.add
            )
            ones_a = stat.tile([128, 1], f32)
            nc.vector.memset(ones_a[:], 1.0 / (B * C))
            part = psum_out.tile([1, 1], f32)
            nc.tensor.matmul(part[:], ones_a[:], acc[:], start=True, stop=True)
            res = stat.tile([1, 1], f32)
            nc.vector.tensor_copy(res[:], part[:])
            nc.sync.dma_start(out=out_d[:, :], in_=res[:])

    nc.compile()
    return nc


def _get_nc():
    if "nc" not in _cache:
        _cache["nc"] = _build_nc()
    return _cache["nc"]


def _get_perm():
    """Constant per-row ascending-argsort of the fixed uniform matrix."""
    if "perm" not in _cache:
        import jax

        with jax.default_device(jax.devices("cpu")[0]):
            u = np.asarray(jax.random.uniform(jax.random.key(42), (B, C)))
        _cache["perm"] = np.argsort(u, axis=1, kind="stable")
    return _cache["perm"]


def _consts():
    if "wtri" not in _cache:
        # lhsT[k,i] = [k<=i], diag -1024  (matmul computes lhsT.T @ rhs)
        w = np.triu(np.ones((128, 128), np.float32))
        np.fill_diagonal(w, -1024.0)
        _cache["wtri"] = w.astype(np.float16)
        i = np.arange(128, dtype=np.float64)[:, None]
        cb = np.arange(NB, dtype=np.float64)[None, :]
        thr = -1280.0 * cb - 10.0 * (i + 1.0) - 511.6
        _cache["thr"] = thr.astype(np.float32)
    return _cache["wtri"], _cache["thr"]


def _make_in_maps(scores: np.ndarray, attributes: np.ndarray):
    perm = _get_perm()
    s_p = np.take_along_axis(np.asarray(scores, dtype=np.float32), perm, axis=1)
    a_p = np.take_along_axis(np.asarray(attributes, dtype=np.int32), perm, axis=1)
    s16 = s_p.astype(np.float16)
    g16 = (10 - 20 * a_p).astype(np.float16)
    wtri, thr = _consts()
    in_maps = []
    for i in range(N_CORES):
        r0, r1 = i * ROWS, (i + 1) * ROWS
        in_maps.append(
            {
                "s": np.ascontiguousarray(s16[r0:r1].T),
                "gg": np.ascontiguousarray(g16[r0:r1].T),
                "wtri": wtri,
                "thr": thr,
            }
        )
    return in_maps


def _run(in_maps, trace=False, **kwargs):
    from concourse import bass_utils

    return bass_utils.run_bass_kernel_spmd(
        _get_nc(), in_maps, core_ids=list(range(N_CORES)), trace=trace, **kwargs
    )


def kernel(scores: np.ndarray, attributes: np.ndarray) -> np.ndarray:
    res = _run(_make_in_maps(scores, attributes))
    parts = np.stack(
        [np.asarray(r["out"], dtype=np.float32).reshape(()) for r in res.results]
    )
    return np.float32(np.sum(parts, dtype=np.float32)).reshape(())[()]


# revision 14
# speedup vs baseline: 24.0532x; 1.0224x over previous
"""Trainium2 Bass kernel for nn_AttrSoftLoss (masked multilabel soft-margin loss).

Reference semantics: per row, drop the k = round(0.95 * n_zero) zero-labeled
positions whose fixed uniform draws (jax.random.key(42)) are smallest, then
average  -[a*log_sigmoid(s) + (1-a)*log_sigmoid(-s)]  over kept positions;
mean over rows.  With g = 1-2a and x = g*s this is
loss = [sum_kept softplus(x)] / (B*C)  (the mask keeps all a=1 positions).

Host prep (layout/encoding only): rows pre-permuted into ascending order of
the fixed input-independent uniform matrix (the dropped set becomes "the
first k zero-labeled entries" in storage order), data stored TRANSPOSED
(classes on partitions, rows on the free dim), scores cast to fp16, labels
recoded as gg = 10*(1-2a) in {+10,-10} fp16 (the 10x lets every derived
count stay integer-exact in fp16 and folds into Exp's scale immediate).

Device math per [128, 1024] class-block cb: the keep decision
c > rint(0.95*nz) (c = inclusive zero-prefix count, nz = row zero count) is
evaluated in the integer-exact scaled form Q = 20c + 20*1025*a - 19*nz -
10.4 > 0 (deviates from round-half-even only on ~234 of 8.4M boundary
elements, rel err 5e-5, numpy-verified).  In gg-units all data terms are
linear, giving per block two PE matmuls and one fused DVE op:

    q_psum = W @ gg_cb + J @ V_cb                      (PE, f32-exact)
    W[k,i] = [k<=i], diag -1024    (own-block prefix + ones-pusher)
    V_cb   = -0.95*GT + sum_{b<cb} gg_b                (DVE fp16 chain; GT =
             sum_b gg_b; all values half-integers, fp16-exact)
    kept  <=> q_psum > thr[i,cb] = -1280*cb - 10*(i+1) - 511.6  (f32 const)
    stt(scr, q_psum, thr_ptr, sp, is_gt, mult, accum_out=stats)

No prefix scan (2.7us/block on DVE, v1) and no GpSimd cross-lane reduce
(127us/block on HW!, v4): cross-block counts ride on fp16 chain adds (2x
DVE mode) contracted by the all-ones J matmul on the PE.

ScalarE computes softplus(x) = Ln(1 + Exp(0.1*xx)), xx = gg*s, over
[128, 2048] chunks; the act-table list passed to insert_act_table_loads is
pruned (order-preserving, so runtime set ids stay valid) so Exp and Ln both
resolve to natural_log_exp_and_others: one table load total.
Batch is sharded 1024 rows/core (pure data parallel); the host sums the 8
partial scalars at gather time (a 4-byte device AllReduce costs ~50us + a
~100us NEFF entry barrier, dominating the whole kernel).
"""

import numpy as np

B, C = 8192, 1024
N_CORES = 8
ROWS = B // N_CORES  # 1024 rows per core (free dim after transpose)
NB = C // 128        # 8 class-blocks per core (partition dim)

_cache: dict = {}


def _make_bacc():
    from concourse import bacc, mybir

    class PrunedTableBacc(bacc.Bacc):
        """Prune Exp/Ln from every act-table set except
        natural_log_exp_and_others (order preserved, so the emitted
        act_func_set_id still indexes the real act_info list) - forces the
        first-fit chooser to put Exp and Ln on the one shared table."""

        def insert_act_table_loads(self):
            import bass_rust as _bass_rust
            from concourse.hw_specs import get_activation_tables

            keep = "natural_log_exp_and_others"
            drop = {
                mybir.ActivationFunctionType.Exp,
                mybir.ActivationFunctionType.Ln,
            }
            tables = []
            for name, funcs in get_activation_tables(self.m.arch).items():
                if name != keep:
                    funcs = {f for f in funcs if f not in drop}
                tables.append((name, funcs))
            _bass_rust.insert_act_table_loads(self, tables)

    return PrunedTableBacc(
        "TRN2", target_bir_lowering=False, debug=False, num_devices=N_CORES
    )


def _build_nc():
    from concourse import mybir, tile

    Alu = mybir.AluOpType
    Act = mybir.ActivationFunctionType
    f32 = mybir.dt.float32
    f16 = mybir.dt.float16

    nc = _make_bacc()
    s_d = nc.dram_tensor("s", [C, ROWS], f16, kind="ExternalInput")
    g_d = nc.dram_tensor("gg", [C, ROWS], f16, kind="ExternalInput")
    w_d = nc.dram_tensor("wtri", [128, 128], f16, kind="ExternalInput")
    thr_d = nc.dram_tensor("thr", [128, NB], f32, kind="ExternalInput")
    out_d = nc.dram_tensor("out", [1, 1], f32, kind="ExternalOutput")

    with tile.TileContext(nc) as tc:
        with (
            tc.tile_pool(name="work", bufs=3) as work,
            tc.tile_pool(name="stat", bufs=1) as stat,
            tc.tile_pool(name="psum", bufs=3, space="PSUM") as psum,
            tc.tile_pool(name="psum_out", bufs=1, space="PSUM") as psum_out,
        ):
            wtri = stat.tile([128, 128], f16)
            thr = stat.tile([128, NB], f32)
            stats = stat.tile([128, NB], f32)
            jmat = stat.tile([128, 128], f16)
            nc.sync.dma_start(out=wtri[:], in_=w_d[:, :])
            nc.sync.dma_start(out=thr[:], in_=thr_d[:, :])
            nc.vector.memset(jmat[:], 1.0)

            g_big = stat.tile([128, NB * ROWS], f16)
            s_big = stat.tile([128, NB * ROWS], f16)
            x_big = stat.tile([128, NB * ROWS], f16)
            ex_big = stat.tile([128, NB * ROWS], f16)
            sp_big = stat.tile([128, NB * ROWS], f16)

            def blk(t, cb):
                return t[:, ROWS * cb : ROWS * (cb + 1)]

            ones_a = stat.tile([128, 1], f32)
            nc.vector.memset(ones_a[:], 1.0 / (B * C))

            # 2:1 gg-weighted DMA weave: gg gates the V0 barrier (needs all
            # blocks), s feeds the ACT stream continuously.
            def dma_g(cb):
                nc.sync.dma_start(
                    out=blk(g_big, cb), in_=g_d[128 * cb : 128 * (cb + 1), :]
                )

            def dma_s(cb):
                nc.sync.dma_start(
                    out=blk(s_big, cb), in_=s_d[128 * cb : 128 * (cb + 1), :]
                )

            for op in [lambda: dma_g(0), lambda: dma_s(0), lambda: dma_g(1),
                       lambda: dma_g(2), lambda: dma_s(1), lambda: dma_g(3),
                       lambda: dma_g(4), lambda: dma_s(2), lambda: dma_g(5),
                       lambda: dma_g(6), lambda: dma_s(3), lambda: dma_g(7),
                       lambda: dma_s(4), lambda: dma_s(5), lambda: dma_s(6),
                       lambda: dma_s(7)]:
                op()

            def x_of(cb):
                nc.vector.tensor_tensor(
                    blk(x_big, cb), blk(g_big, cb), blk(s_big, cb), Alu.mult
                )

            def sp_of(cb):
                nc.scalar.activation(
                    blk(ex_big, cb), blk(x_big, cb), Act.Exp, scale=0.1
                )
                nc.scalar.activation(
                    blk(sp_big, cb), blk(ex_big, cb), Act.Ln, bias=1.0
                )

            # DVE stream in arrival order: x products as s lands, GT pair
            # tree as gg lands, then V0 and the V chain.
            p01 = stat.tile([128, ROWS], f16)
            p23 = stat.tile([128, ROWS], f16)
            p45 = stat.tile([128, ROWS], f16)
            p67 = stat.tile([128, ROWS], f16)
            p03 = stat.tile([128, ROWS], f16)
            p47 = stat.tile([128, ROWS], f16)
            gt = stat.tile([128, ROWS], f16)
            x_of(0)
            sp_of(0)
            nc.vector.tensor_tensor(p01[:], blk(g_big, 0), blk(g_big, 1), Alu.add)
            x_of(1)
            sp_of(1)
            nc.vector.tensor_tensor(p23[:], blk(g_big, 2), blk(g_big, 3), Alu.add)
            x_of(2)
            sp_of(2)
            nc.vector.tensor_tensor(p45[:], blk(g_big, 4), blk(g_big, 5), Alu.add)
            x_of(3)
            sp_of(3)
            nc.vector.tensor_tensor(p67[:], blk(g_big, 6), blk(g_big, 7), Alu.add)
            nc.vector.tensor_tensor(p03[:], p01[:], p23[:], Alu.add)
            nc.vector.tensor_tensor(p47[:], p45[:], p67[:], Alu.add)
            nc.vector.tensor_tensor(gt[:], p03[:], p47[:], Alu.add)
            V = [None] * NB
            v0 = stat.tile([128, ROWS], f16, tag="V0")
            nc.vector.tensor_scalar(v0[:], gt[:], -0.95, None, Alu.mult)
            V[0] = v0
            x_of(4)
            sp_of(4)
            for cb in range(1, NB):
                nxt = stat.tile([128, ROWS], f16, tag=f"V{cb}")
                nc.vector.tensor_tensor(
                    nxt[:], V[cb - 1][:], blk(g_big, cb - 1), Alu.add
                )
                V[cb] = nxt
                if cb + 4 < NB:
                    x_of(cb + 4)
                    sp_of(cb + 4)

            # PE: own-block W matmuls for tiles 0-2 first (they can run long
            # before V0 exists); J matmuls + stt pipelined with depth 3.
            qs = [None] * NB

            def w_mms(cb):
                qs[cb] = psum.tile([128, ROWS], f32, tag="q", name=f"q{cb}")
                for h in range(2):
                    sl = slice(512 * h, 512 * (h + 1))
                    nc.tensor.matmul(
                        qs[cb][:, sl], wtri[:],
                        g_big[:, ROWS * cb + 512 * h : ROWS * cb + 512 * (h + 1)],
                        start=True, stop=False,
                    )

            def j_mms(cb):
                for h in range(2):
                    sl = slice(512 * h, 512 * (h + 1))
                    nc.tensor.matmul(
                        qs[cb][:, sl], jmat[:], V[cb][:, sl],
                        start=False, stop=True,
                    )

            def stt(cb):
                scr = work.tile([128, ROWS], f16, tag="scr")
                nc.vector.scalar_tensor_tensor(
                    scr[:], qs[cb][:], thr[:, cb : cb + 1], blk(sp_big, cb),
                    op0=Alu.is_gt, op1=Alu.mult,
                    accum_out=stats[:, cb : cb + 1],
                )

            w_mms(0)
            w_mms(1)
            w_mms(2)
            for cb in range(NB):
                j_mms(cb)
                if cb + 3 < NB:
                    w_mms(cb + 3)
                stt(cb)

            acc = stat.tile([128, 1], f32)
            nc.vector.tensor_reduce(
                acc[:], stats[:], mybir.AxisListType.X, Alu.add
            )
            part = psum_out.tile([1, 1], f32)
            nc.tensor.matmul(part[:], ones_a[:], acc[:], start=True, stop=True)
            res = stat.tile([1, 1], f32)
            nc.vector.tensor_copy(res[:], part[:])
            nc.sync.dma_start(out=out_d[:, :], in_=res[:])

    nc.compile()
    return nc


def _get_nc():
    if "nc" not in _cache:
        _cache["nc"] = _build_nc()
    return _cache["nc"]


def _get_perm():
    """Constant per-row ascending-argsort of the fixed uniform matrix."""
    if "perm" not in _cache:
        import jax

        with jax.default_device(jax.devices("cpu")[0]):
            u = np.asarray(jax.random.uniform(jax.random.key(42), (B, C)))
        _cache["perm"] = np.argsort(u, axis=1, kind="stable")
    return _cache["perm"]


def _consts():
    if "wtri" not in _cache:
        # lhsT[k,i] = [k<=i], diag -1024  (matmul computes lhsT.T @ rhs)
        w = np.triu(np.ones((128, 128), np.float32))
        np.fill_diagonal(w, -1024.0)
        _cache["wtri"] = w.astype(np.float16)
        i = np.arange(128, dtype=np.float64)[:, None]
        cb = np.arange(NB, dtype=np.float64)[None, :]
        thr = -1280.0 * cb - 10.0 * (i + 1.0) - 511.6
        _cache["thr"] = thr.astype(np.float32)
    return _cache["wtri"], _cache["thr"]


def _make_in_maps(scores: np.ndarray, attributes: np.ndarray):
    perm = _get_perm()
    s_p = np.take_along_axis(np.asarray(scores, dtype=np.float32), perm, axis=1)
    a_p = np.take_along_axis(np.asarray(attributes, dtype=np.int32), perm, axis=1)
    s16 = s_p.astype(np.float16)
    g16 = (10 - 20 * a_p).astype(np.float16)
    wtri, thr = _consts()
    in_maps = []
    for i in range(N_CORES):
        r0, r1 = i * ROWS, (i + 1) * ROWS
        in_maps.append(
            {
                "s": np.ascontiguousarray(s16[r0:r1].T),
                "gg": np.ascontiguousarray(g16[r0:r1].T),
                "wtri": wtri,
                "thr": thr,
            }
        )
    return in_maps


def _run(in_maps, trace=False, **kwargs):
    from concourse import bass_utils

    return bass_utils.run_bass_kernel_spmd(
        _get_nc(), in_maps, core_ids=list(range(N_CORES)), trace=trace, **kwargs
    )


def kernel(scores: np.ndarray, attributes: np.ndarray) -> np.ndarray:
    res = _run(_make_in_maps(scores, attributes))
    parts = np.stack(
        [np.asarray(r["out"], dtype=np.float32).reshape(()) for r in res.results]
    )
    return np.float32(np.sum(parts, dtype=np.float32)).reshape(())[()]
